# revision 12
# baseline (speedup 1.0000x reference)
"""DoubleFeatureTransformerSlice — Trainium2 Bass kernel.

out_s[b, :] = bias + sum_k values_s[b, k] * weight[indices_s[b, k], :]   (s = 0, 1)

Sharding: data-parallel over batch across 8 NeuronCores; weight replicated
(converted to fp16 on host).  Each core handles 1024 rows of slice0 + 1024
rows of slice1 = 16 tiles of 128 samples x K=32 (idx, val) pairs.

Kernel design (MODE "v5", HW-validated by repeat-slope this session):
  - Weight table in fp16 (absmax rel err ~8.4e-4 vs f32 reference; gate 2e-2).
  - Gathers via SWDGE dma_gather, 1024 rows (8 k-slots) per call, 2 KB
    descriptors.  Measured gather-only floor 389 us/core (~345 GB/s/core);
    2 KB descriptors are the efficiency sweet spot: fp8 1 KB descriptors
    measured SLOWER (506 us) due to a sub-2KB per-descriptor penalty, and
    per-(tile,k) indirect DMAs (128-row calls) are descriptor/Pool-bound at
    ~1.34 us/call (688 us/core).  One SWDGE queue (2 queues measured slower).
    dma_gather calls must stay <= 1024 descriptors (SWDGE ring) — a
    2048-row call wedges the device (mesh desync).
  - Compute split across three lanes per k-slot so accumulation overlaps the
    gather stream (the DVE STT chain is an exposed serial cost — STT has no
    DVE 2x/4x perf mode, ~1.35 us per op):
      k == 0          -> DVE scalar_tensor_tensor seeds acc_fp16 = g*v + bias.
      k in ACT_KS (4) -> ACT engine t = g * v (Copy w/ scale AP), then DVE
                         tensor_tensor acc += t (fp16 2x mode, ~0.7 us).
      else (27 slots) -> PE: psum_f32 += diag(v_k) @ g_k, diag [128,128] fp16
                         built on DVE tensor_scalar from an identity (~160 ns).
    Per-tile merge: out_f32 = psum + acc (DVE tensor_tensor), DMA out.
    Lane-split A/Bs: all-PE is pstate-bound (~728 us); STT-heavy is
    chain-bound (608+ us); 4 STT + 28 PE measured 388-545; 3 ACT slots
    gained ~27 us over that (matches the 1.35->0.7 us/slot serial-chain
    model); 4 ACT slots (validated slope 380 us, and ~16 us/rep faster than
    3 ACT in a same-dispatch-mode interleaved A/B) ships.  8 ACT slots is
    slightly worse (tt-adds chain on DVE).  Deep pools (psum 4, acc 6)
    pipeline tiles.

Measured HW (repeat-slope, min-of-N, f32 anchor 841 us ~= grader baseline
818 us): this config 353 us end-to-end via test.py (valid same-mode R=8/14
pair) — at/below the dma_gather-only floor probe (389 us, loaded window);
also 519 (3 ACT) vs 545 (4 STT + 28 PE) vs 531 (8 ACT + 23 PE) in a loaded
validated window.  dma_gather single_packet default (True) confirmed
equal-or-better than False.  gath_bufs=8 (16 MB gather staging) beat 6 by
~74 us/rep in a same-dispatch-mode replica A/B (two jit instances of the
same program can land in different dispatch modes — compare same-mode mins
only) and by ~110 us in adjacent validated-slope windows; correctness
identical (4.9e-4).
Caveat: the PJRT/axon fixed dispatch cost is bimodal per dispatch burst
(~42 vs ~76 ms), so slopes are only valid when both R points land in the
same mode (test.py validates+retries); absolute numbers drift 380-650 us
with terminal load.

MODE "v7" (ships, session 2) = v5 + out written fp16, host upcasts to f32
(output rel err 6.4e-4 vs 4.9e-4; gate 2e-2).  Rationale: the kernel is
end-to-end DMA-bus-bound (gather 128MB + out + inputs ~ 140MB/core at
~345GB/s ~= 406us ~= graded 415us), so the only remaining lever was output
bytes: 8MB -> 4MB per core, ~ -11us steady-state.  Confirmed directionally
in a loaded window (min 537 vs 543.5, median paired delta -25us).
Session-2 probes that closed off everything bigger (same-window A/B,
loaded units):
  - fp8 1KB-descriptor gather: 534us vs fp16-2KB 494us -> the sub-2KB
    per-descriptor penalty is REAL; all 1-byte-dtype ideas are dead
    (e3m4 would also pass at 1.24e-2, moot).  int8+global-scale passes
    at 3.9e-3 but PE matmul takes float dtypes only.
  - fp16 4KB descriptors (2 rows/desc, same bytes): 340us (x0.69) -- but
    random indices can't be paired into adjacent vocab rows (expected
    useful pairs per core ~90 of 33k needed), so unusable.
  - SBUF-resident table + GPSIMD ap_gather: ~0.83ns/elem/partition/0.6
    -> ~730us/core for the 524288 elem/partition expansion.  Dead.
  - dedup (3.1x within core, 23x global): expansion back to sample-major
    tiles needs the same 65536 per-row descriptors -- the descriptor
    count, not gathered bytes, is the binding constraint.  Dead.
  - dense one-hot PE matmul vs the V=22528 table: 94.5 GFLOP/core ~2ms.
IMPORTANT measurement note: the R-repeat slope only sees steady-state
per-repeat cost; prologue (idx/val/bias loads, ~5-10us) is invisible to
it but likely counted by the grader's full-NEFF exec time.  idx_chunks
(split idx loads to start gathers earlier) measured worst-in-window twice
at chunks=8 -> left at 1.
"""

import numpy as np

MODE = "v7"

NCORES = 8
B = 8192
K = 32
D = 1024
V = 22528
P = 128
BPC = B // NCORES          # batch rows per core per slice
ROWS = 2 * BPC             # rows per core (slice0 chunk + slice1 chunk)
NTILES = ROWS // P         # 16 tiles of 128 samples

_cached = {}
LAST_RESULTS = None        # BassKernelResults of the last run (for harness)


def _build_v5(repeats: int = 1, gath_bufs: int = 8, accp_bufs: int = 6,
              psum_bufs: int = 4, gpg: int = 8, act_ks=(4, 12, 20, 28)):
    import concourse.bacc as bacc
    import concourse.mybir as mybir
    import concourse.tile as tile
    from concourse.masks import make_identity

    nidx = gpg * P             # rows per dma_gather call (<= 1024!)
    cpg = nidx // 16           # idx16 columns per call
    ncalls_tile = K // gpg
    act_ks = tuple(act_ks)
    nc = bacc.Bacc(
        "TRN2",
        target_bir_lowering=False,
        debug=False,
        enable_asserts=False,
        num_devices=NCORES,
    )
    w = nc.dram_tensor("w", [V, D], mybir.dt.float16, kind="ExternalInput")
    idx16 = nc.dram_tensor(
        "idx16", [P, NTILES * ncalls_tile * cpg], mybir.dt.int16, kind="ExternalInput"
    )
    val = nc.dram_tensor("val", [ROWS, K], mybir.dt.float32, kind="ExternalInput")
    bias = nc.dram_tensor("bias_bcast", [P, D], mybir.dt.float16, kind="ExternalInput")
    out = nc.dram_tensor("out", [ROWS, D], mybir.dt.float32, kind="ExternalOutput")

    with tile.TileContext(nc) as tc:
        with (
            tc.tile_pool(name="gath", bufs=gath_bufs) as gpool,
            tc.tile_pool(name="accp", bufs=accp_bufs) as apool,
            tc.tile_pool(name="tmul", bufs=4) as tpool,
            tc.tile_pool(name="diag", bufs=8) as dpool,
            tc.tile_pool(name="psum", bufs=psum_bufs, space="PSUM") as ppool,
            tc.tile_pool(name="outs", bufs=3) as opool,
            tc.tile_pool(name="const", bufs=1) as cpool,
        ):
            bias_t = cpool.tile([P, D], mybir.dt.float16, tag="bias")
            nc.sync.dma_start(bias_t[:], bias[:, :])
            ident = cpool.tile([P, P], mybir.dt.float16, tag="ident")
            make_identity(nc, ident[:])
            idxs = cpool.tile(
                [P, NTILES * ncalls_tile * cpg], mybir.dt.int16, tag="ix"
            )
            nc.sync.dma_start(idxs[:], idx16[:, :])
            val_all = cpool.tile([P, NTILES, K], mybir.dt.float32, tag="vala")
            nc.sync.dma_start(val_all[:], val[:, :].rearrange("(t p) k -> p t k", p=P))
            for t in range(NTILES * repeats):
                t = t % NTILES
                r0 = t * P
                val_t = val_all[:, t]
                acc = apool.tile([P, D], mybir.dt.float16, tag="acc")
                psum = ppool.tile([P, D], mybir.dt.float32, tag="ps")
                pe_ks = [k for k in range(K) if k != 0 and k not in act_ks]
                for gi in range(ncalls_tile):
                    gid = t * ncalls_tile + gi
                    g = gpool.tile([P, gpg, D], mybir.dt.float16, tag="g")
                    nc.gpsimd.dma_gather(
                        g[:],
                        w[:, :],
                        idxs[:, gid * cpg : (gid + 1) * cpg],
                        nidx,
                        nidx,
                        D,
                    )
                    for j in range(gpg):
                        k = gi * gpg + j
                        if k == 0:
                            nc.vector.scalar_tensor_tensor(
                                out=acc[:],
                                in0=g[:, j, :],
                                scalar=val_t[:, k : k + 1],
                                in1=bias_t[:],
                                op0=mybir.AluOpType.mult,
                                op1=mybir.AluOpType.add,
                            )
                        elif k in act_ks:
                            tm = tpool.tile([P, D], mybir.dt.float16, tag="tm")
                            nc.scalar.activation(
                                out=tm[:],
                                in_=g[:, j, :],
                                func=mybir.ActivationFunctionType.Copy,
                                scale=val_t[:, k : k + 1],
                            )
                            nc.vector.tensor_tensor(
                                out=acc[:], in0=tm[:], in1=acc[:],
                                op=mybir.AluOpType.add,
                            )
                        else:
                            diag = dpool.tile([P, P], mybir.dt.float16, tag="dg")
                            nc.vector.tensor_scalar(
                                out=diag[:],
                                in0=ident[:],
                                scalar1=val_t[:, k : k + 1],
                                scalar2=None,
                                op0=mybir.AluOpType.mult,
                            )
                            first, last = k == pe_ks[0], k == pe_ks[-1]
                            nc.tensor.matmul(
                                out=psum[:, 0:512], lhsT=diag[:], rhs=g[:, j, 0:512],
                                start=first, stop=last,
                            )
                            nc.tensor.matmul(
                                out=psum[:, 512:1024], lhsT=diag[:],
                                rhs=g[:, j, 512:1024],
                                start=first, stop=last,
                            )
                outt = opool.tile([P, D], mybir.dt.float32, tag="o")
                nc.vector.tensor_tensor(
                    out=outt[:], in0=psum[:], in1=acc[:], op=mybir.AluOpType.add
                )
                nc.sync.dma_start(out[r0 : r0 + P, :], outt[:])
    nc.compile()
    return nc


def _build_v7(repeats: int = 1, gath_bufs: int = 8, accp_bufs: int = 6,
              psum_bufs: int = 4, gpg: int = 8, act_ks=(4, 12, 20, 28),
              idx_chunks: int = 1):
    """v5 + out written fp16 (host upcasts to f32): 8MB -> 4MB per core of
    output writes on the same DMA bus the gathers saturate (~11us/rep).
    idx_chunks>1 (split idx loads for earlier first gather) measured
    worst-in-window twice at chunks=8 -> default 1.  gath_bufs=10 with SBUF
    at 200/208KB also measured slower -> keep 8."""
    import concourse.bacc as bacc
    import concourse.mybir as mybir
    import concourse.tile as tile
    from concourse.masks import make_identity

    nidx = gpg * P
    cpg = nidx // 16
    ncalls_tile = K // gpg
    ncalls = NTILES * ncalls_tile
    act_ks = tuple(act_ks)
    assert ncalls % idx_chunks == 0
    cpc = ncalls // idx_chunks          # gather calls per idx chunk
    nc = bacc.Bacc(
        "TRN2",
        target_bir_lowering=False,
        debug=False,
        enable_asserts=False,
        num_devices=NCORES,
    )
    w = nc.dram_tensor("w", [V, D], mybir.dt.float16, kind="ExternalInput")
    idx16 = nc.dram_tensor(
        "idx16", [P, ncalls * cpg], mybir.dt.int16, kind="ExternalInput"
    )
    val = nc.dram_tensor("val", [ROWS, K], mybir.dt.float32, kind="ExternalInput")
    bias = nc.dram_tensor("bias_bcast", [P, D], mybir.dt.float16, kind="ExternalInput")
    out = nc.dram_tensor("out", [ROWS, D], mybir.dt.float16, kind="ExternalOutput")

    with tile.TileContext(nc) as tc:
        with (
            tc.tile_pool(name="gath", bufs=gath_bufs) as gpool,
            tc.tile_pool(name="accp", bufs=accp_bufs) as apool,
            tc.tile_pool(name="tmul", bufs=4) as tpool,
            tc.tile_pool(name="diag", bufs=8) as dpool,
            tc.tile_pool(name="psum", bufs=psum_bufs, space="PSUM") as ppool,
            tc.tile_pool(name="outs", bufs=3) as opool,
            tc.tile_pool(name="const", bufs=1) as cpool,
        ):
            idxc = []
            for ci in range(idx_chunks):
                ic = cpool.tile([P, cpc * cpg], mybir.dt.int16, tag=f"ix{ci}")
                nc.sync.dma_start(ic[:], idx16[:, ci * cpc * cpg:(ci + 1) * cpc * cpg])
                idxc.append(ic)
            bias_t = cpool.tile([P, D], mybir.dt.float16, tag="bias")
            nc.sync.dma_start(bias_t[:], bias[:, :])
            ident = cpool.tile([P, P], mybir.dt.float16, tag="ident")
            make_identity(nc, ident[:])
            val_all = cpool.tile([P, NTILES, K], mybir.dt.float32, tag="vala")
            nc.sync.dma_start(val_all[:], val[:, :].rearrange("(t p) k -> p t k", p=P))
            for t in range(NTILES * repeats):
                t = t % NTILES
                r0 = t * P
                val_t = val_all[:, t]
                acc = apool.tile([P, D], mybir.dt.float16, tag="acc")
                psum = ppool.tile([P, D], mybir.dt.float32, tag="ps")
                pe_ks = [k for k in range(K) if k != 0 and k not in act_ks]
                for gi in range(ncalls_tile):
                    gid = t * ncalls_tile + gi
                    ci, co = gid // cpc, gid % cpc
                    g = gpool.tile([P, gpg, D], mybir.dt.float16, tag="g")
                    nc.gpsimd.dma_gather(
                        g[:],
                        w[:, :],
                        idxc[ci][:, co * cpg : (co + 1) * cpg],
                        nidx,
                        nidx,
                        D,
                    )
                    for j in range(gpg):
                        k = gi * gpg + j
                        if k == 0:
                            nc.vector.scalar_tensor_tensor(
                                out=acc[:],
                                in0=g[:, j, :],
                                scalar=val_t[:, k : k + 1],
                                in1=bias_t[:],
                                op0=mybir.AluOpType.mult,
                                op1=mybir.AluOpType.add,
                            )
                        elif k in act_ks:
                            tm = tpool.tile([P, D], mybir.dt.float16, tag="tm")
                            nc.scalar.activation(
                                out=tm[:],
                                in_=g[:, j, :],
                                func=mybir.ActivationFunctionType.Copy,
                                scale=val_t[:, k : k + 1],
                            )
                            nc.vector.tensor_tensor(
                                out=acc[:], in0=tm[:], in1=acc[:],
                                op=mybir.AluOpType.add,
                            )
                        else:
                            diag = dpool.tile([P, P], mybir.dt.float16, tag="dg")
                            nc.vector.tensor_scalar(
                                out=diag[:],
                                in0=ident[:],
                                scalar1=val_t[:, k : k + 1],
                                scalar2=None,
                                op0=mybir.AluOpType.mult,
                            )
                            first, last = k == pe_ks[0], k == pe_ks[-1]
                            nc.tensor.matmul(
                                out=psum[:, 0:512], lhsT=diag[:], rhs=g[:, j, 0:512],
                                start=first, stop=last,
                            )
                            nc.tensor.matmul(
                                out=psum[:, 512:1024], lhsT=diag[:],
                                rhs=g[:, j, 512:1024],
                                start=first, stop=last,
                            )
                outt = opool.tile([P, D], mybir.dt.float16, tag="o")
                nc.vector.tensor_tensor(
                    out=outt[:], in0=psum[:], in1=acc[:], op=mybir.AluOpType.add
                )
                nc.sync.dma_start(out[r0 : r0 + P, :], outt[:])
    nc.compile()
    return nc


def _build_v6(repeats: int = 1, gath_bufs: int = 12, accp_bufs: int = 6,
              psum_bufs: int = 4, gpg: int = 8,
              act_ks=(0, 3, 6, 9, 12, 15, 18, 21, 24, 27)):
    """fp8(e3m4) weight table: 1KB gather descriptors halve HBM gather bytes.

    Scale folded host-side into val (val' = val/S, w8 = e3m4(w*S)), so device
    math is unchanged: out = bias + sum_k val'_k * w8[idx_k].  Lanes:
      k in act_ks -> ACT tm = g*v' (Copy w/ scale, fp8 in -> fp16 out), DVE
                     tensor_tensor acc += tm (k==act_ks[0] seeds acc = tm+bias)
      else        -> PE: psum += diag(v'_k) @ g_k, diag fp16, rhs fp8e3.
    More ACT slots than v5 (10 vs 4): at a ~190us gather floor, 27 PE slots
    (~184us warm, worse if pstate-throttled) would be critical-path; 22 PE +
    10 ACT keeps every compute engine under ~160us.
    """
    import concourse.bacc as bacc
    import concourse.mybir as mybir
    import concourse.tile as tile
    from concourse.masks import make_identity

    nidx = gpg * P
    cpg = nidx // 16
    ncalls_tile = K // gpg
    act_ks = tuple(act_ks)
    assert 0 in act_ks
    nc = bacc.Bacc(
        "TRN2",
        target_bir_lowering=False,
        debug=False,
        enable_asserts=False,
        num_devices=NCORES,
    )
    w = nc.dram_tensor("w", [V, D], mybir.dt.float8e3, kind="ExternalInput")
    idx16 = nc.dram_tensor(
        "idx16", [P, NTILES * ncalls_tile * cpg], mybir.dt.int16, kind="ExternalInput"
    )
    val = nc.dram_tensor("val", [ROWS, K], mybir.dt.float32, kind="ExternalInput")
    bias = nc.dram_tensor("bias_bcast", [P, D], mybir.dt.float16, kind="ExternalInput")
    out = nc.dram_tensor("out", [ROWS, D], mybir.dt.float32, kind="ExternalOutput")

    with tile.TileContext(nc) as tc:
        with (
            tc.tile_pool(name="gath", bufs=gath_bufs) as gpool,
            tc.tile_pool(name="accp", bufs=accp_bufs) as apool,
            tc.tile_pool(name="tmul", bufs=4) as tpool,
            tc.tile_pool(name="diag", bufs=8) as dpool,
            tc.tile_pool(name="psum", bufs=psum_bufs, space="PSUM") as ppool,
            tc.tile_pool(name="outs", bufs=3) as opool,
            tc.tile_pool(name="const", bufs=1) as cpool,
        ):
            bias_t = cpool.tile([P, D], mybir.dt.float16, tag="bias")
            nc.sync.dma_start(bias_t[:], bias[:, :])
            ident = cpool.tile([P, P], mybir.dt.float16, tag="ident")
            make_identity(nc, ident[:])
            idxs = cpool.tile(
                [P, NTILES * ncalls_tile * cpg], mybir.dt.int16, tag="ix"
            )
            nc.sync.dma_start(idxs[:], idx16[:, :])
            val_all = cpool.tile([P, NTILES, K], mybir.dt.float32, tag="vala")
            nc.sync.dma_start(val_all[:], val[:, :].rearrange("(t p) k -> p t k", p=P))
            for t in range(NTILES * repeats):
                t = t % NTILES
                r0 = t * P
                val_t = val_all[:, t]
                acc = apool.tile([P, D], mybir.dt.float16, tag="acc")
                psum = ppool.tile([P, D], mybir.dt.float32, tag="ps")
                pe_ks = [k for k in range(K) if k not in act_ks]
                for gi in range(ncalls_tile):
                    gid = t * ncalls_tile + gi
                    g = gpool.tile([P, gpg, D], mybir.dt.float8e3, tag="g")
                    nc.gpsimd.dma_gather(
                        g[:],
                        w[:, :],
                        idxs[:, gid * cpg : (gid + 1) * cpg],
                        nidx,
                        nidx,
                        D,
                    )
                    for j in range(gpg):
                        k = gi * gpg + j
                        if k in act_ks:
                            tm = tpool.tile([P, D], mybir.dt.float16, tag="tm")
                            nc.scalar.activation(
                                out=tm[:],
                                in_=g[:, j, :],
                                func=mybir.ActivationFunctionType.Copy,
                                scale=val_t[:, k : k + 1],
                            )
                            if k == act_ks[0]:
                                nc.vector.tensor_tensor(
                                    out=acc[:], in0=tm[:], in1=bias_t[:],
                                    op=mybir.AluOpType.add,
                                )
                            else:
                                nc.vector.tensor_tensor(
                                    out=acc[:], in0=tm[:], in1=acc[:],
                                    op=mybir.AluOpType.add,
                                )
                        else:
                            diag = dpool.tile([P, P], mybir.dt.float16, tag="dg")
                            nc.vector.tensor_scalar(
                                out=diag[:],
                                in0=ident[:],
                                scalar1=val_t[:, k : k + 1],
                                scalar2=None,
                                op0=mybir.AluOpType.mult,
                            )
                            first, last = k == pe_ks[0], k == pe_ks[-1]
                            nc.tensor.matmul(
                                out=psum[:, 0:512], lhsT=diag[:], rhs=g[:, j, 0:512],
                                start=first, stop=last,
                            )
                            nc.tensor.matmul(
                                out=psum[:, 512:1024], lhsT=diag[:],
                                rhs=g[:, j, 512:1024],
                                start=first, stop=last,
                            )
                outt = opool.tile([P, D], mybir.dt.float32, tag="o")
                nc.vector.tensor_tensor(
                    out=outt[:], in0=psum[:], in1=acc[:], op=mybir.AluOpType.add
                )
                nc.sync.dma_start(out[r0 : r0 + P, :], outt[:])
    nc.compile()
    return nc


def _build(repeats: int = 1, mode: str | None = None, **kw):
    mode = mode or MODE
    if mode.startswith("v6"):
        return _build_v6(repeats, **kw)
    if mode.startswith("v7"):
        return _build_v7(repeats, **kw)
    return _build_v5(repeats, **kw)


def _wrap_idx16(idx_c: np.ndarray, gpg: int = 8) -> np.ndarray:
    """[ROWS, K] int -> [P, ncalls * nidx/16] int16 in dma_gather's wrap-16
    layout (index i of a call lives at [i % 16, i // 16]; row i = j*128 + p
    feeds out[:, j, :] partition p; pattern replicated across partitions)."""
    nidx = gpg * P
    ncpt = K // gpg
    A = idx_c.reshape(NTILES, P, ncpt, gpg)
    cols = []
    for t in range(NTILES):
        for gi in range(ncpt):
            flat = A[t, :, gi, :].T.reshape(-1)          # i = j*128 + p
            cols.append(flat.reshape(nidx // 16, 16).T)  # [16, nidx/16]
    w16 = np.concatenate(cols, axis=1)
    return np.ascontiguousarray(np.tile(w16, (P // 16, 1)).astype(np.int16))


def prep_in_maps(fi0, fv0, fi1, fv1, weight, bias, mode=None, gpg: int = 8):
    mode = mode or MODE
    b = np.asarray(bias, dtype=np.float16)
    bias_b = np.ascontiguousarray(np.broadcast_to(b[None, :], (P, D)))
    wf = np.asarray(weight).astype(np.float32)
    if mode.startswith("v6"):
        import ml_dtypes

        # e3m4 table, scaled so |w*S| ~ 4 (e3m4 max 15.5); fold 1/S into val
        S = 4.0 / max(float(np.abs(wf).max()), 1e-30)
        w = np.ascontiguousarray((wf * S).astype(ml_dtypes.float8_e3m4))
        vscale = np.float32(1.0 / S)
    else:
        w = np.ascontiguousarray(wf.astype(np.float16))
        vscale = np.float32(1.0)
    in_maps = []
    for c in range(NCORES):
        sl = slice(c * BPC, (c + 1) * BPC)
        idx_c = np.concatenate([fi0[sl], fi1[sl]], axis=0)
        val_c = np.ascontiguousarray(
            np.concatenate([fv0[sl], fv1[sl]], axis=0).astype(np.float32) * vscale
        )
        in_maps.append(
            {"w": w, "val": val_c, "bias_bcast": bias_b,
             "idx16": _wrap_idx16(idx_c.astype(np.int64), gpg=gpg)}
        )
    return in_maps


def kernel(
    feature_indices_0,
    feature_values_0,
    feature_indices_1,
    feature_values_1,
    weight,
    bias,
):
    global LAST_RESULTS
    from concourse.bass_utils import run_bass_kernel_spmd

    if MODE not in _cached:
        _cached[MODE] = _build()
    nc = _cached[MODE]

    in_maps = prep_in_maps(
        np.asarray(feature_indices_0),
        np.asarray(feature_values_0),
        np.asarray(feature_indices_1),
        np.asarray(feature_values_1),
        weight,
        bias,
        MODE,
    )
    try:
        res = run_bass_kernel_spmd(nc, in_maps, core_ids=list(range(NCORES)))
    except ModuleNotFoundError:
        # BASS_TRACE set but this axon client lacks the NTFF profile hook
        # (antenv.axon_hooks) — rerun with tracing disabled.
        import os

        os.environ["BASS_NEVER_TRACE"] = "1"
        res = run_bass_kernel_spmd(nc, in_maps, core_ids=list(range(NCORES)))
    LAST_RESULTS = res
    outs = [np.asarray(r["out"], dtype=np.float32) for r in res.results]
    out0 = np.concatenate([o[:BPC] for o in outs], axis=0)
    out1 = np.concatenate([o[BPC:] for o in outs], axis=0)
    return (out0, out1)

